# revision 14
# baseline (speedup 1.0000x reference)
"""Masked (expander) linear layer on 8 Trainium2 NeuronCores.

Computes out = x @ (W * M)^T for
  x: [16384, 2048] f32, W: [2048, 2048] f32, M: [2048, 2048] int32 (0/1)

Sharding: pure data-parallel over rows of x. Each of the 8 cores gets 2048
rows of x plus a replicated (transposed) copy of W and M, computes its
[2048, 2048] output shard entirely locally (mask-multiply on DVE, matmul on
PE), and the host concatenates shards. No collectives.

Device-side design:
 - All tensors are laid out on host *piece-major* so every steady-state
   DMA reads contiguous 2-4KB per partition line (small-line DMAs are
   packet-rate-limited): W/M/x become [piece][128, piece_cols] with the
   piece's (k-tile, column) panel flattened per partition. The
   contraction dim lands on SBUF partitions.
 - W, x ride in bf16 (rel-err budget is 2e-2; bf16 operands with f32
   PSUM accumulation land ~2e-3) at the same 1 cycle/row PE rate as
   f32r. The mask rides as int8; the mask-multiply runs on DVE, one
   instruction per piece.
 - x (8MB) is fully SBUF-resident next to the masked weight (8.4MB),
   and the matmul sweep is n-panel-major: phase nt uses only W panel nt
   (2.1MB), so the W/mask streams are never tight — the DMA rings share
   one 16-engine pool with per-packet round-robin, so no single stream
   can count on more than ~1/3 of the ~350GB/s aggregate while others
   are active. Demand per stream stays well under that share.
 - The first W/x pieces are issued k-tile-fine so the PE starts once
   ~128KB has landed, and a junk warm-up matmul group runs during the
   DMA head so the Tensor engine's p-state ramp is spent before the
   first real matmul.
 - PSUM groups (one m-tile x 512 outputs) rotate over 8 banks with
   evacuation (Act copy + scalar-ring DMA) inlined right after each
   group closes; the final group set drains at half-tile grain to
   shorten the tail.
"""

from contextlib import ExitStack

import ml_dtypes
import numpy as np

import concourse.bacc as bacc
import concourse.bass as bass
import concourse.mybir as mybir
import concourse.tile as tile
from concourse.bass_utils import run_bass_kernel_spmd

N_CORES = 8
P = 128

FULL_N, FULL_OUT, FULL_IN = 16384, 2048, 2048

MASK_DTYPES = {
    "int8": (mybir.dt.int8, np.int8),
    "bfloat16": (mybir.dt.bfloat16, ml_dtypes.bfloat16),
    "float32": (mybir.dt.float32, np.float32),
}

OPERAND_DTYPES = {
    "bfloat16": (mybir.dt.bfloat16, ml_dtypes.bfloat16),
    "float32": (mybir.dt.float32, np.float32),
}


def build_nc(
    rows: int = FULL_N // N_CORES,
    in_dim: int = FULL_IN,
    out_dim: int = FULL_OUT,
    op_dtype: str = "bfloat16",
    mask_dtype: str = "int8",
    n_chunk: int = 512,
    m_block: int = 4,
    warmup_mms: int = 8,
):
    """Per-core Bass module: y[rows, out] = x @ (wt * m).

    DRAM layouts (piece-major, contiguous per piece):
      wt/mk: [NT, 4, P, KQ*n_chunk]  piece (nt, q), per-partition [kt, n]
      x:     [NB, 4, P, KQ*mw]       piece (b, q),  per-partition [kt, m]
      y:     [rows, out_dim] row-major f32.
    """
    assert rows % P == 0 and in_dim % P == 0 and out_dim % n_chunk == 0
    KT = in_dim // P
    MT = rows // P
    NT = out_dim // n_chunk
    assert KT % 4 == 0 and MT % m_block == 0
    KQ = KT // 4
    NB = MT // m_block
    mw = m_block * P  # columns of x per block

    mdt, _ = MASK_DTYPES[mask_dtype]
    odt, _ = OPERAND_DTYPES[op_dtype]

    nc = bacc.Bacc("TRN2", target_bir_lowering=False, debug=False)
    x = nc.dram_tensor("x", [NB, 4, P, KQ * mw], odt, kind="ExternalInput")
    wt = nc.dram_tensor("wt", [NT, 4, P, KQ * n_chunk], odt, kind="ExternalInput")
    mk = nc.dram_tensor("mk", [NT, 4, P, KQ * n_chunk], mdt, kind="ExternalInput")
    y = nc.dram_tensor("y", [rows, out_dim], mybir.dt.float32, kind="ExternalOutput")

    # piece views: [.., p, kt, cols]
    wt_v = wt[:, :, :, :].rearrange("t q p (k n) -> t q p k n", k=KQ)
    mk_v = mk[:, :, :, :].rearrange("t q p (k n) -> t q p k n", k=KQ)
    x_v = x[:, :, :, :].rearrange("b q p (k m) -> b q p k m", k=KQ)

    with ExitStack() as ctx:
        tc = ctx.enter_context(tile.TileContext(nc))
        wm_pool = ctx.enter_context(tc.tile_pool(name="wm", bufs=1))
        ws_pool = ctx.enter_context(tc.tile_pool(name="ws", bufs=3))
        msk_pool = ctx.enter_context(tc.tile_pool(name="msk", bufs=3))
        xt_pool = ctx.enter_context(tc.tile_pool(name="xt", bufs=1))
        yo_pool = ctx.enter_context(tc.tile_pool(name="yo", bufs=3))
        wu_pool = ctx.enter_context(tc.tile_pool(name="wu", bufs=1))
        pm_pool = ctx.enter_context(tc.tile_pool(name="pm", bufs=1, space="PSUM"))

        # Resident masked weight: wm_t[nt][q] of shape [P, KQ, n_chunk]
        wm_t = [
            [
                wm_pool.tile(
                    [P, KQ, n_chunk], odt, tag=f"wm{nt}_{q}", name=f"wm{nt}_{q}"
                )
                for q in range(4)
            ]
            for nt in range(NT)
        ]
        # Resident x: tiles [P, KQ, mw] per (block, k-quarter)
        x_t = [
            [
                xt_pool.tile([P, KQ, mw], odt, tag=f"xt{b}_{q}", name=f"xt{b}_{q}")
                for q in range(4)
            ]
            for b in range(NB)
        ]

        # ---- PE p-state warm-up: junk matmuls while the first DMAs fly ----
        if warmup_mms:
            wu = wu_pool.tile([P, n_chunk], odt, tag="wu", name="wu")
            nc.gpsimd.memset(wu[:], 0.0)
            # one accumulation group on the bank that real group 7 will use
            # much later; results are never read.
            pmw = pm_pool.tile([P, n_chunk], mybir.dt.float32, tag="pm7", name="pmw")
            for i in range(warmup_mms):
                nc.tensor.matmul(
                    pmw[:],
                    wu[:, :P],
                    wu[:],
                    start=(i == 0),
                    stop=(i == warmup_mms - 1),
                )

        def load_w_piece(nt, q, fine=False):
            # W rides the sync HWDGE ring; masks ride the scalar ring.
            # fine=True splits the piece per k-tile so the first matmul can
            # start after ~one k-tile (128KB) instead of a full quarter.
            wstage = ws_pool.tile([P, KQ, n_chunk], odt, tag="ws")
            mtile = msk_pool.tile([P, KQ, n_chunk], mdt, tag="mt")
            if fine:
                for k in range(KQ):
                    nc.sync.dma_start(out=wstage[:, k, :], in_=wt_v[nt, q, :, k, :])
                    nc.scalar.dma_start(out=mtile[:, k, :], in_=mk_v[nt, q, :, k, :])
                    nc.vector.tensor_mul(
                        wm_t[nt][q][:, k, :], wstage[:, k, :], mtile[:, k, :]
                    )
            else:
                nc.sync.dma_start(out=wstage[:], in_=wt_v[nt, q])
                nc.scalar.dma_start(out=mtile[:], in_=mk_v[nt, q])
                nc.vector.tensor_mul(wm_t[nt][q][:], wstage[:], mtile[:])

        def load_x_piece(b, q, fine=False):
            # x has the SWDGE ring to itself.
            if fine:
                for k in range(KQ):
                    nc.gpsimd.dma_start(out=x_t[b][q][:, k, :], in_=x_v[b, q, :, k, :])
            else:
                nc.gpsimd.dma_start(out=x_t[b][q][:], in_=x_v[b, q])

        # ---- prep: all of x on the SWDGE ring, W/mask pieces on their rings,
        # both in first-consumed-first order ----
        for b in range(NB):
            for q in range(4):
                load_x_piece(b, q, fine=(b == 0 and q == 0))
        for nt in range(NT):
            for q in range(4):
                load_w_piece(nt, q, fine=(nt == 0 and q == 0))

        # ---- main: n-panel phases; inside, blocks of m_block m-tiles ----
        for nt in range(NT):
            last_phase = nt == NT - 1
            for b in range(NB):
                last_group_set = last_phase and b == NB - 1
                # rotating PSUM banks: group g frees its bank 8 groups later
                pms = {
                    mb: pm_pool.tile(
                        [P, n_chunk],
                        mybir.dt.float32,
                        tag=f"pm{(b * m_block + mb) % 8}",
                        name=f"pm{(b * m_block + mb) % 8}",
                    )
                    for mb in range(m_block)
                }
                # k-quarter-outer: each sub-group only needs its own pieces
                for q in range(4):
                    for mb in range(m_block):
                        for k in range(KQ):
                            kt = q * KQ + k
                            nc.tensor.matmul(
                                pms[mb][:],
                                x_t[b][q][:, k, bass.ts(mb, P)],
                                wm_t[nt][q][:, k, :],
                                start=(kt == 0),
                                stop=(kt == KT - 1),
                            )
                        if q == 3:
                            # evacuate as soon as this group closes
                            mt = b * m_block + mb
                            yrow = y[mt * P : (mt + 1) * P, bass.ts(nt, n_chunk)]
                            yo = yo_pool.tile(
                                [P, n_chunk], mybir.dt.float32, tag="yo"
                            )
                            if last_group_set:
                                # finer drain: halve copy/DMA grains so the
                                # post-last-matmul critical path shrinks
                                hn = n_chunk // 2
                                nc.scalar.copy(yo[:, :hn], pms[mb][:, :hn])
                                nc.scalar.dma_start(
                                    out=yrow[:, :hn], in_=yo[:, :hn]
                                )
                                nc.scalar.copy(yo[:, hn:], pms[mb][:, hn:])
                                nc.scalar.dma_start(
                                    out=yrow[:, hn:], in_=yo[:, hn:]
                                )
                            else:
                                nc.scalar.copy(yo[:], pms[mb][:])
                                nc.scalar.dma_start(out=yrow, in_=yo[:])

    nc.compile()
    return nc


def _prep_host(
    input_, weight, mask, op_dtype="bfloat16", mask_dtype="int8", n_chunk=512,
    m_block=4,
):
    _, npmdt = MASK_DTYPES[mask_dtype]
    _, npodt = OPERAND_DTYPES[op_dtype]
    in_dim, out_dim = weight.shape[1], weight.shape[0]
    NT = out_dim // n_chunk
    KT = in_dim // P
    KQ = KT // 4
    mw = m_block * P

    def pack_panels(mat_t, nt, ncols, dtype):
        # mat_t: [in_dim, cols]; -> [nt, 4, P, KQ*ncols] piece-major
        a = mat_t.reshape(4, KQ, P, nt, ncols)
        return np.ascontiguousarray(
            a.transpose(3, 0, 2, 1, 4).reshape(nt, 4, P, KQ * ncols)
        ).astype(dtype)

    wtp = pack_panels(np.asarray(weight).T, NT, n_chunk, npodt)
    mkp = pack_panels(np.asarray(mask).T, NT, n_chunk, npmdt)
    rows = input_.shape[0] // N_CORES
    NB = rows // mw
    in_maps = []
    for c in range(N_CORES):
        xs = input_[c * rows : (c + 1) * rows]  # [rows, in_dim]
        xp = pack_panels(xs.T, NB, mw, npodt)
        in_maps.append({"x": xp, "wt": wtp, "mk": mkp})
    return in_maps


_CACHE = {}


def _run(input_, weight, mask, trace=False, **build_kw):
    rows_total, in_dim = input_.shape
    out_dim = weight.shape[0]
    key = (rows_total, in_dim, out_dim, tuple(sorted(build_kw.items())))
    if key not in _CACHE:
        _CACHE[key] = build_nc(
            rows=rows_total // N_CORES, in_dim=in_dim, out_dim=out_dim, **build_kw
        )
    nc = _CACHE[key]
    in_maps = _prep_host(
        input_,
        weight,
        mask,
        build_kw.get("op_dtype", "bfloat16"),
        build_kw.get("mask_dtype", "int8"),
        build_kw.get("n_chunk", 512),
        build_kw.get("m_block", 4),
    )
    res = run_bass_kernel_spmd(nc, in_maps, core_ids=list(range(N_CORES)), trace=trace)
    out = np.concatenate([res.results[c]["y"] for c in range(N_CORES)], axis=0)
    return out, res


def kernel(input_, weight, mask):
    input_ = np.asarray(input_, dtype=np.float32)
    weight = np.asarray(weight, dtype=np.float32)
    mask = np.asarray(mask)
    out, _ = _run(input_, weight, mask, trace=False)
    return out


# revision 17
# speedup vs baseline: 1.0423x; 1.0423x over previous
"""Masked (expander) linear layer on 8 Trainium2 NeuronCores.

Computes out = x @ (W * M)^T for
  x: [16384, 2048] f32, W: [2048, 2048] f32, M: [2048, 2048] int32 (0/1)

Sharding: pure data-parallel over rows of x. Each of the 8 cores gets 2048
rows of x plus a replicated (transposed) copy of W and M, computes its
[2048, 2048] output shard entirely locally (mask-multiply on DVE, matmul on
PE), and the host concatenates shards. No collectives.

Device-side design:
 - All tensors are laid out on host *piece-major* so every steady-state
   DMA reads contiguous 2-4KB per partition line (small-line DMAs are
   packet-rate-limited): W/M/x become [piece][128, piece_cols] with the
   piece's (k-tile, column) panel flattened per partition. The
   contraction dim lands on SBUF partitions.
 - W, x ride in bf16 (rel-err budget is 2e-2; bf16 operands with f32
   PSUM accumulation land ~2e-3) at the same 1 cycle/row PE rate as
   f32r. The mask rides as int8; the mask-multiply runs on DVE, one
   instruction per piece.
 - x (8MB) is fully SBUF-resident next to the masked weight (8.4MB),
   and the matmul sweep is n-panel-major: phase nt uses only W panel nt
   (2.1MB), so the W/mask streams are never tight — the DMA rings share
   one 16-engine pool with per-packet round-robin, so no single stream
   can count on more than ~1/3 of the ~350GB/s aggregate while others
   are active. Demand per stream stays well under that share.
 - The first W/x pieces are issued k-tile-fine so the PE starts once
   ~128KB has landed, and a junk warm-up matmul group runs during the
   DMA head so the Tensor engine's p-state ramp is spent before the
   first real matmul.
 - PSUM groups (one m-tile x 512 outputs) rotate over 8 banks with
   evacuation (Act copy + scalar-ring DMA) inlined right after each
   group closes; the final group set drains at half-tile grain to
   shorten the tail.
"""

from contextlib import ExitStack

import ml_dtypes
import numpy as np

import concourse.bacc as bacc
import concourse.bass as bass
import concourse.mybir as mybir
import concourse.tile as tile
from concourse.bass_utils import run_bass_kernel_spmd

N_CORES = 8
P = 128

FULL_N, FULL_OUT, FULL_IN = 16384, 2048, 2048

MASK_DTYPES = {
    "int8": (mybir.dt.int8, np.int8),
    "bfloat16": (mybir.dt.bfloat16, ml_dtypes.bfloat16),
    "float32": (mybir.dt.float32, np.float32),
}

OPERAND_DTYPES = {
    "bfloat16": (mybir.dt.bfloat16, ml_dtypes.bfloat16),
    "float32": (mybir.dt.float32, np.float32),
}


def build_nc(
    rows: int = FULL_N // N_CORES,
    in_dim: int = FULL_IN,
    out_dim: int = FULL_OUT,
    op_dtype: str = "bfloat16",
    mask_dtype: str = "int8",
    n_chunk: int = 512,
    m_block: int = 4,
    warmup_mms: int = 8,
):
    """Per-core Bass module: y[rows, out] = x @ (wt * m).

    DRAM layouts (piece-major, contiguous per piece):
      wt/mk: [NT, 4, P, KQ*n_chunk]  piece (nt, q), per-partition [kt, n]
      x:     [NB, 4, P, KQ*mw]       piece (b, q),  per-partition [kt, m]
      y:     [rows, out_dim] row-major f32.
    """
    assert rows % P == 0 and in_dim % P == 0 and out_dim % n_chunk == 0
    KT = in_dim // P
    MT = rows // P
    NT = out_dim // n_chunk
    assert KT % 4 == 0 and MT % m_block == 0
    KQ = KT // 4
    NB = MT // m_block
    mw = m_block * P  # columns of x per block

    mdt, _ = MASK_DTYPES[mask_dtype]
    odt, _ = OPERAND_DTYPES[op_dtype]

    nc = bacc.Bacc("TRN2", target_bir_lowering=False, debug=False)
    x = nc.dram_tensor("x", [NB, 4, P, KQ * mw], odt, kind="ExternalInput")
    wt = nc.dram_tensor("wt", [NT, 4, P, KQ * n_chunk], odt, kind="ExternalInput")
    mk = nc.dram_tensor("mk", [NT, 4, P, KQ * n_chunk], mdt, kind="ExternalInput")
    y = nc.dram_tensor("y", [rows, out_dim], mybir.dt.float32, kind="ExternalOutput")

    # piece views: [.., p, kt, cols]
    wt_v = wt[:, :, :, :].rearrange("t q p (k n) -> t q p k n", k=KQ)
    mk_v = mk[:, :, :, :].rearrange("t q p (k n) -> t q p k n", k=KQ)
    x_v = x[:, :, :, :].rearrange("b q p (k m) -> b q p k m", k=KQ)

    with ExitStack() as ctx:
        tc = ctx.enter_context(tile.TileContext(nc))
        wm_pool = ctx.enter_context(tc.tile_pool(name="wm", bufs=1))
        ws_pool = ctx.enter_context(tc.tile_pool(name="ws", bufs=3))
        msk_pool = ctx.enter_context(tc.tile_pool(name="msk", bufs=3))
        xt_pool = ctx.enter_context(tc.tile_pool(name="xt", bufs=1))
        yo_pool = ctx.enter_context(tc.tile_pool(name="yo", bufs=4))
        wu_pool = ctx.enter_context(tc.tile_pool(name="wu", bufs=1))
        pm_pool = ctx.enter_context(tc.tile_pool(name="pm", bufs=1, space="PSUM"))

        # Resident masked weight: wm_t[nt][q] of shape [P, KQ, n_chunk]
        wm_t = [
            [
                wm_pool.tile(
                    [P, KQ, n_chunk], odt, tag=f"wm{nt}_{q}", name=f"wm{nt}_{q}"
                )
                for q in range(4)
            ]
            for nt in range(NT)
        ]
        # Resident x: tiles [P, KQ, mw] per (block, k-quarter)
        x_t = [
            [
                xt_pool.tile([P, KQ, mw], odt, tag=f"xt{b}_{q}", name=f"xt{b}_{q}")
                for q in range(4)
            ]
            for b in range(NB)
        ]

        # ---- PE p-state warm-up: junk matmuls while the first DMAs fly ----
        if warmup_mms:
            wu = wu_pool.tile([P, n_chunk], odt, tag="wu", name="wu")
            nc.gpsimd.memset(wu[:], 0.0)
            # one accumulation group on the bank that real group 7 will use
            # much later; results are never read.
            pmw = pm_pool.tile([P, n_chunk], mybir.dt.float32, tag="pm7", name="pmw")
            for i in range(warmup_mms):
                nc.tensor.matmul(
                    pmw[:],
                    wu[:, :P],
                    wu[:],
                    start=(i == 0),
                    stop=(i == warmup_mms - 1),
                )

        def load_w_piece(nt, q, eng, fine=False):
            # masks always ride the scalar ring; the W DMA rides `eng`.
            # fine=True splits the piece per k-tile so the first matmul can
            # start after ~one k-tile (128KB) instead of a full quarter.
            wstage = ws_pool.tile([P, KQ, n_chunk], odt, tag="ws")
            mtile = msk_pool.tile([P, KQ, n_chunk], mdt, tag="mt")
            if fine:
                for k in range(KQ):
                    eng.dma_start(out=wstage[:, k, :], in_=wt_v[nt, q, :, k, :])
                    nc.scalar.dma_start(out=mtile[:, k, :], in_=mk_v[nt, q, :, k, :])
                    nc.vector.tensor_mul(
                        wm_t[nt][q][:, k, :], wstage[:, k, :], mtile[:, k, :]
                    )
            else:
                eng.dma_start(out=wstage[:], in_=wt_v[nt, q])
                nc.scalar.dma_start(out=mtile[:], in_=mk_v[nt, q])
                nc.vector.tensor_mul(wm_t[nt][q][:], wstage[:], mtile[:])

        def load_x_piece(b, q, eng, fine=False):
            if fine:
                for k in range(KQ):
                    eng.dma_start(out=x_t[b][q][:, k, :], in_=x_v[b, q, :, k, :])
            else:
                eng.dma_start(out=x_t[b][q][:], in_=x_v[b, q])

        # ---- prep: pieces issued in first-consumed order, alternating the
        # two bulk HWDGE/SWDGE queues so no single queue must sustain the
        # phase-0 block-0 double stream (x + W in lockstep) ----
        A, B = nc.gpsimd, nc.sync
        for q in range(4):
            ex, ew = (A, B) if q % 2 == 0 else (B, A)
            load_x_piece(0, q, ex, fine=(q == 0))
            load_w_piece(0, q, ew, fine=(q == 0))
        flip = 0
        for b in range(1, NB):
            for q in range(4):
                load_x_piece(b, q, (A, B)[flip % 2])
                flip += 1
        for nt in range(1, NT):
            for q in range(4):
                load_w_piece(nt, q, (A, B)[flip % 2])
                flip += 1

        # ---- main: n-panel phases; inside, blocks of m_block m-tiles ----
        for nt in range(NT):
            last_phase = nt == NT - 1
            for b in range(NB):
                last_group_set = last_phase and b == NB - 1
                # rotating PSUM banks: group g frees its bank 8 groups later
                pms = {
                    mb: pm_pool.tile(
                        [P, n_chunk],
                        mybir.dt.float32,
                        tag=f"pm{(b * m_block + mb) % 8}",
                        name=f"pm{(b * m_block + mb) % 8}",
                    )
                    for mb in range(m_block)
                }
                # k-quarter-outer: each sub-group only needs its own pieces
                for q in range(4):
                    for mb in range(m_block):
                        for k in range(KQ):
                            kt = q * KQ + k
                            nc.tensor.matmul(
                                pms[mb][:],
                                x_t[b][q][:, k, bass.ts(mb, P)],
                                wm_t[nt][q][:, k, :],
                                start=(kt == 0),
                                stop=(kt == KT - 1),
                            )
                        if q == 3:
                            # evacuate as soon as this group closes
                            mt = b * m_block + mb
                            yrow = y[mt * P : (mt + 1) * P, bass.ts(nt, n_chunk)]
                            yo = yo_pool.tile(
                                [P, n_chunk], mybir.dt.float32, tag="yo"
                            )
                            if last_group_set:
                                # two parallel drain pipes (Act+scalar ring /
                                # DVE+sync ring) at half-tile grain so the
                                # post-last-matmul critical path shrinks
                                hn = n_chunk // 2
                                if mb % 2 == 0:
                                    nc.scalar.copy(yo[:, :hn], pms[mb][:, :hn])
                                    nc.scalar.dma_start(
                                        out=yrow[:, :hn], in_=yo[:, :hn]
                                    )
                                    nc.scalar.copy(yo[:, hn:], pms[mb][:, hn:])
                                    nc.scalar.dma_start(
                                        out=yrow[:, hn:], in_=yo[:, hn:]
                                    )
                                else:
                                    nc.vector.tensor_copy(
                                        yo[:, :hn], pms[mb][:, :hn]
                                    )
                                    nc.sync.dma_start(
                                        out=yrow[:, :hn], in_=yo[:, :hn]
                                    )
                                    nc.vector.tensor_copy(
                                        yo[:, hn:], pms[mb][:, hn:]
                                    )
                                    nc.sync.dma_start(
                                        out=yrow[:, hn:], in_=yo[:, hn:]
                                    )
                            else:
                                nc.scalar.copy(yo[:], pms[mb][:])
                                nc.scalar.dma_start(out=yrow, in_=yo[:])

    nc.compile()
    return nc


def _prep_host(
    input_, weight, mask, op_dtype="bfloat16", mask_dtype="int8", n_chunk=512,
    m_block=4,
):
    _, npmdt = MASK_DTYPES[mask_dtype]
    _, npodt = OPERAND_DTYPES[op_dtype]
    in_dim, out_dim = weight.shape[1], weight.shape[0]
    NT = out_dim // n_chunk
    KT = in_dim // P
    KQ = KT // 4
    mw = m_block * P

    def pack_panels(mat_t, nt, ncols, dtype):
        # mat_t: [in_dim, cols]; -> [nt, 4, P, KQ*ncols] piece-major
        a = mat_t.reshape(4, KQ, P, nt, ncols)
        return np.ascontiguousarray(
            a.transpose(3, 0, 2, 1, 4).reshape(nt, 4, P, KQ * ncols)
        ).astype(dtype)

    wtp = pack_panels(np.asarray(weight).T, NT, n_chunk, npodt)
    mkp = pack_panels(np.asarray(mask).T, NT, n_chunk, npmdt)
    rows = input_.shape[0] // N_CORES
    NB = rows // mw
    in_maps = []
    for c in range(N_CORES):
        xs = input_[c * rows : (c + 1) * rows]  # [rows, in_dim]
        xp = pack_panels(xs.T, NB, mw, npodt)
        in_maps.append({"x": xp, "wt": wtp, "mk": mkp})
    return in_maps


_CACHE = {}


def _run(input_, weight, mask, trace=False, **build_kw):
    rows_total, in_dim = input_.shape
    out_dim = weight.shape[0]
    key = (rows_total, in_dim, out_dim, tuple(sorted(build_kw.items())))
    if key not in _CACHE:
        _CACHE[key] = build_nc(
            rows=rows_total // N_CORES, in_dim=in_dim, out_dim=out_dim, **build_kw
        )
    nc = _CACHE[key]
    in_maps = _prep_host(
        input_,
        weight,
        mask,
        build_kw.get("op_dtype", "bfloat16"),
        build_kw.get("mask_dtype", "int8"),
        build_kw.get("n_chunk", 512),
        build_kw.get("m_block", 4),
    )
    res = run_bass_kernel_spmd(nc, in_maps, core_ids=list(range(N_CORES)), trace=trace)
    out = np.concatenate([res.results[c]["y"] for c in range(N_CORES)], axis=0)
    return out, res


def kernel(input_, weight, mask):
    input_ = np.asarray(input_, dtype=np.float32)
    weight = np.asarray(weight, dtype=np.float32)
    mask = np.asarray(mask)
    out, _ = _run(input_, weight, mask, trace=False)
    return out
